# revision 58
# baseline (speedup 1.0000x reference)
"""Trainium2 Bass kernel for GroupNorm + multi-head self-attention block.

Reference computation (per batch element):
    xn  = GroupNorm(x; 32 groups, eps=1e-5) * norm_w + norm_b
    qkv = qkv_w @ xn + qkv_b          (1x1 conv == channel matmul)
    q,k,v split; 4 heads of dh=128 over 1024 spatial positions
    attn = softmax(q^T k * C**-0.5); out = attn @ v
    out = proj_w @ out + proj_b + xn

Sharding: pure data-parallel over batch (16 batches / 8 cores = 2 per core),
no collectives.

Precision / matmul modes:
  - qkv, attn*v + softmax-denominator, and proj matmuls all run in fp8e4m3
    DoubleRow perf mode (two 128-deep contraction tiles per instruction =
    2x PE throughput); only the K=128 score matmuls stay bf16 (DoubleRow
    has no advantage at contraction 128).
  - Weights are prescaled x16 on the host (fp8 sweet range); inverse scales
    fold into free evac scale factors. ones8 = 1/16 makes rc = 16/sum(e), so
    attnout lands at 16x scale for fp8 and proj evac divides by 256.
  - softmax exp is split per i-half: Scalar engine native Exp -> fp8 for
    half 0 (plus half 1 of j-tiles 0 and 4, rebalancing the busier Vector
    engine), Vector engine Schraudolph bit-trick (affine + truncating uint8
    convert, bitcast fp8e4m3, B calibrated 56.05) for the rest of half 1.
  - v-bias is folded into proj_b on the host (softmax weights sum to 1).
  - GroupNorm statistics fp32 (pooling via small fp32r indicator matmuls,
    batched across all 4 channel tiles); residual in bf16; output f32.
  - Attention-path fp8/Schraudolph noise (~3%) is benign: attnout is only
    ~3.5% of the output norm (the xn residual dominates).

Schedule highlights:
  - x DMAs are emitted before the weight-constant DMAs (single SP DMA queue)
    so GroupNorm stats start ~6us earlier.
  - GroupNorm runs per batch so batch 0's qkv matmuls start while batch 1's
    stats are still on the Vector engine; xn_bf (residual-only) is produced
    on the otherwise-idle GpSimd engine; xn_f8 lands on ACT for ct0/1 (they
    gate the first qkv DoubleRow pair) and DVE for ct2/3 (batch 0), GpSimd
    for batch 1.
  - Attention is software-pipelined: dn/ot DoubleRow accumulation trails the
    score matmuls by one j-tile pair so exp latency stays off the PE
    critical path.
"""

from contextlib import ExitStack

import numpy as np

B = 16          # full batch
C = 512         # channels
S = 1024        # spatial (32*32)
HEADS = 4
DH = C // HEADS         # 128, head dim == partition tile
GROUPS = 32
EPS = 1e-5
NCORES = 8
BPC = B // NCORES       # 2 batches per core
CT = C // 128           # 4 channel tiles
SCALE = float(C) ** -0.5
LOG2E = 1.4426950408889634
A_SCH = 8.0 * LOG2E            # fp8e4m3 bits per unit exp-argument
B_SCH = 56.05                  # calibrated for DVE's truncating u8 convert
JT = S // 128           # 8 j-tiles (key positions)
NH = S // 512           # 2 free-dim halves

_CACHE = {}


def _emit(tc, io):
    from concourse import mybir

    nc = tc.nc
    f32 = mybir.dt.float32
    f32r = mybir.dt.float32r
    bf16 = mybir.dt.bfloat16
    f8 = mybir.dt.float8e4
    u8 = mybir.dt.uint8
    Act = mybir.ActivationFunctionType
    Alu = mybir.AluOpType
    PM = mybir.MatmulPerfMode

    x_d = io["x"]
    out_d = io["out"]

    with ExitStack() as ctx:
        consts = ctx.enter_context(tc.tile_pool(name="consts", bufs=1))
        x_pool = ctx.enter_context(tc.tile_pool(name="x_pool", bufs=8))
        xnbf_pool = ctx.enter_context(tc.tile_pool(name="xnbf_pool", bufs=1))
        stats = ctx.enter_context(tc.tile_pool(name="stats", bufs=4))
        qk_pool = ctx.enter_context(tc.tile_pool(name="qk_pool", bufs=2))
        vt_pool = ctx.enter_context(tc.tile_pool(name="vt_pool", bufs=2))
        ao_pool = ctx.enter_context(tc.tile_pool(name="ao_pool", bufs=2))
        e_pool = ctx.enter_context(tc.tile_pool(name="e_pool", bufs=8))
        rc_pool = ctx.enter_context(tc.tile_pool(name="rc_pool", bufs=4))
        fo_pool = ctx.enter_context(tc.tile_pool(name="fo_pool", bufs=6))
        # PSUM pools: shared mm/scores(4) + o(2) + dn(2) = 8 banks
        mm1 = ctx.enter_context(tc.tile_pool(name="mm1", bufs=4, space="PSUM"))
        o_ps = ctx.enter_context(tc.tile_pool(name="o_ps", bufs=2, space="PSUM"))
        dn_ps = ctx.enter_context(tc.tile_pool(name="dn_ps", bufs=2, space="PSUM"))

        # ---- constants (emission deferred: x DMAs must go first on the
        # single SP DMA queue; GN-phase consts precede gn_rest, the heavy
        # weight DMAs follow) ----
        qkvT8 = consts.tile([128, CT, 3 * C], f8, name="qkvT8")
        projT8 = consts.tile([128, CT, C], f8, name="projT8")
        qkvb_sb = consts.tile([128, 12], f32, name="qkvb_sb")
        gnw_sb = consts.tile([128, CT], f32, name="gnw_sb")
        gnb_sb = consts.tile([128, CT], f32, name="gnb_sb")
        projb_sb = consts.tile([128, CT], f32, name="projb_sb")
        indp_sb = consts.tile([128, 8], f32r, name="indp_sb")
        indb_sb = consts.tile([8, 128], f32r, name="indb_sb")
        ones8 = consts.tile([128, 2, 128], f8, name="ones8")
        eps_sb = consts.tile([8, 1], f32, name="eps_sb")

        def emit_consts_gn():
            nc.sync.dma_start(out=indp_sb, in_=io["indp"])
            nc.sync.dma_start(out=indb_sb, in_=io["indb"])
            nc.sync.dma_start(out=gnw_sb, in_=io["gnw"])
            nc.sync.dma_start(out=gnb_sb, in_=io["gnb"])
            nc.vector.memset(ones8, 1.0 / 16.0)
            nc.vector.memset(eps_sb, EPS)

        def emit_consts_weights():
            nc.sync.dma_start(out=qkvT8, in_=io["qkvT8"])
            nc.sync.dma_start(out=projT8, in_=io["projT8"])
            nc.sync.dma_start(out=qkvb_sb, in_=io["qkvb"])
            nc.sync.dma_start(out=projb_sb, in_=io["projb"])

        # normalized x per batch: bf16 for the residual/scores path, fp8 for
        # the DoubleRow qkv matmuls
        xn_bf = [
            xnbf_pool.tile([128, BPC, S], bf16, name=f"xnbf{k}") for k in range(CT)
        ]
        xn_f8 = xnbf_pool.tile([128, CT, BPC, S], f8, name="xn_f8")

        gn_state = {}

        def emit_gn_stats(b):
            """GroupNorm per-channel stats for batch b (Vector engine only)."""
            sb_st = stats.tile([128, CT, 4], f32r, name="sb_st")
            sb_stf = stats.tile([128, CT, 4], f32, name="sb_stf")
            xts = []
            for k in range(CT):
                xt = x_pool.tile([128, S], f32, name="xt")
                nc.sync.dma_start(out=xt, in_=x_d[b, k * 128:(k + 1) * 128, :])
                bn6 = stats.tile([128, 2, 6], f32, name="bn6")
                for u in range(2):
                    nc.vector.bn_stats(
                        out=bn6[:, u, :], in_=xt[:, u * 512:(u + 1) * 512]
                    )
                nc.vector.bn_aggr(out=sb_stf[:, k, 0:2], in_=bn6)
                xts.append(xt)
            # mean^2 and duplicated mean for all tiles in one op each
            nc.vector.tensor_mul(sb_stf[:, :, 2:3], sb_stf[:, :, 0:1],
                                 sb_stf[:, :, 0:1])
            nc.vector.tensor_copy(out=sb_stf[:, :, 3:4], in_=sb_stf[:, :, 0:1])
            nc.vector.tensor_copy(out=sb_st, in_=sb_stf)
            gn_state[b] = (xts, sb_st)

        def emit_gn_rest(b, norm_on_act):
            """Group pooling + broadcast + normalize for batch b, all channel
            tiles batched through the pooling matmuls and scalar fixups."""
            xts, sb_st = gn_state.pop(b)
            # pool over 16-channel groups (x 1/16): pg[g, ct, {mean,var,mean2}]
            pgt = mm1.tile([128, 512], f32, name="gn_ps", tag="mm")
            pg = pgt[0:8, 0:CT * 4]
            nc.tensor.matmul(pg, lhsT=indp_sb, rhs=sb_st, start=True, stop=True)
            pgs = stats.tile([8, CT, 4], f32, name="pgs")
            nc.vector.tensor_copy(out=pgs, in_=pg)
            # g_sb cols: [mean_g, rstd_g] per ct
            g_sb = stats.tile([8, CT, 2], f32r, name="g_sb")
            tmp8 = stats.tile([8, CT, 2], f32, name="tmp8")
            nc.vector.tensor_copy(out=g_sb[:, :, 0:1], in_=pgs[:, :, 0:1])
            nc.vector.tensor_mul(tmp8[:, :, 0:1], pgs[:, :, 0:1], pgs[:, :, 0:1])
            nc.vector.tensor_add(tmp8[:, :, 1:2], pgs[:, :, 1:2], pgs[:, :, 2:3])
            nc.vector.tensor_sub(tmp8[:, :, 1:2], tmp8[:, :, 1:2], tmp8[:, :, 0:1])
            nc.scalar.activation(
                out=g_sb[:, :, 1:2], in_=tmp8[:, :, 1:2], func=Act.Sqrt,
                bias=eps_sb,
            )
            with nc.allow_low_precision("fp22 matmul input rounding"):
                nc.vector.reciprocal(out=g_sb[:, :, 1:2], in_=g_sb[:, :, 1:2])
            # broadcast group stats to channels: bc [128, ct, {mean, rstd}]
            bct = mm1.tile([128, 512], f32, name="gn_ps", tag="mm")
            bc = bct[:, 0:CT * 2]
            nc.tensor.matmul(bc, lhsT=indb_sb, rhs=g_sb, start=True, stop=True)
            # sc cols: [posbias, scale] per ct;  xn = x*scale + posbias.
            # All channel tiles fixed up in 3 strided ops: scale = rstd*gamma,
            # posbias = beta - mean*scale.
            sc = stats.tile([128, CT, 2], f32, name="sc")
            nc.vector.tensor_mul(sc[:, :, 1:2], bct[:, 1:2 * CT:2], gnw_sb)
            nc.vector.tensor_mul(sc[:, :, 0:1], bct[:, 0:2 * CT:2],
                                 sc[:, :, 1:2])
            nc.vector.tensor_sub(sc[:, :, 0:1], gnb_sb, sc[:, :, 0:1])
            for k in range(CT):
                if norm_on_act:
                    if k < 2:
                        nc.scalar.activation(
                            out=xn_f8[:, k, b, :],
                            in_=xts[k],
                            func=Act.Identity,
                            bias=sc[:, k, 0:1],
                            scale=sc[:, k, 1:2],
                        )
                    else:
                        nc.vector.tensor_scalar(
                            xn_f8[:, k, b, :],
                            xts[k],
                            sc[:, k, 1:2],
                            sc[:, k, 0:1],
                            op0=Alu.mult,
                            op1=Alu.add,
                        )
                    nc.gpsimd.tensor_scalar(
                        xn_bf[k][:, b, :],
                        xts[k],
                        sc[:, k, 1:2],
                        sc[:, k, 0:1],
                        op0=Alu.mult,
                        op1=Alu.add,
                    )
                else:
                    nc.gpsimd.tensor_scalar(
                        xn_f8[:, k, b, :],
                        xts[k],
                        sc[:, k, 1:2],
                        sc[:, k, 0:1],
                        op0=Alu.mult,
                        op1=Alu.add,
                    )
                    nc.vector.tensor_scalar(
                        xn_bf[k][:, b, :],
                        xts[k],
                        sc[:, k, 1:2],
                        sc[:, k, 0:1],
                        op0=Alu.mult,
                        op1=Alu.add,
                    )

        q_sb = {}
        k_sb = {}
        vt_sb = {}
        ao_sb = {}

        def emit_qkv(b, evac_on_act=False):
            # q, k: [128, head, 1024]; m-tile 0..3 -> q head, 4..7 -> k head
            q_sb[b] = qk_pool.tile([128, HEADS, S], bf16, name="q_sb")
            k_sb[b] = qk_pool.tile([128, HEADS, S], bf16, name="k_sb")
            for m in range(2 * HEADS):
                dst = q_sb[b] if m < HEADS else k_sb[b]
                for n in range(NH):
                    ps = mm1.tile([128, 512], f32, name="qk_ps", tag="mm")
                    for u in range(2):
                        nc.tensor.matmul(
                            ps,
                            lhsT=qkvT8[:, 2 * u:2 * u + 2, m * 128:(m + 1) * 128],
                            rhs=xn_f8[:, 2 * u:2 * u + 2, b, n * 512:(n + 1) * 512],
                            start=(u == 0),
                            stop=(u == 1),
                            perf_mode=PM.DoubleRow,
                        )
                    dslice = dst[:, m % HEADS, n * 512:(n + 1) * 512]
                    if (m + n) % 2 == 0:
                        nc.scalar.activation(
                            out=dslice, in_=ps, func=Act.Identity,
                            bias=qkvb_sb[:, m:m + 1], scale=1.0 / 16.0,
                        )
                    else:
                        nc.vector.tensor_scalar(
                            dslice, ps, 1.0 / 16.0, qkvb_sb[:, m:m + 1],
                            op0=Alu.mult, op1=Alu.add,
                        )
            # v_T: [128(j), jt, 512(cv)]
            vt_sb[b] = vt_pool.tile([128, JT, C], f8, name="vt_sb")
            for jt in range(JT):
                ps = mm1.tile([128, 512], f32, name="qk_ps", tag="mm")
                for u in range(2):
                    nc.tensor.matmul(
                        ps,
                        lhsT=xn_f8[:, 2 * u:2 * u + 2, b, jt * 128:(jt + 1) * 128],
                        rhs=qkvT8[:, 2 * u:2 * u + 2, 2 * C:3 * C],
                        start=(u == 0),
                        stop=(u == 1),
                        perf_mode=PM.DoubleRow,
                    )
                if jt % 2 == 0:
                    nc.scalar.activation(
                        out=vt_sb[b][:, jt, :], in_=ps, func=Act.Copy,
                        scale=1.0 / 16.0,
                    )
                else:
                    nc.vector.tensor_scalar(
                        vt_sb[b][:, jt, :], ps, 1.0 / 16.0, None, op0=Alu.mult,
                    )

        def emit_attn(b):
            ao_sb[b] = ao_pool.tile([128, HEADS, S], f8, name="ao_sb")
            for h in range(HEADS):
                dns = [dn_ps.tile([128, 512], f32, name="dn") for _ in range(NH)]
                ots = [o_ps.tile([128, 512], f32, name="ot") for _ in range(NH)]
                e2s = [e_pool.tile([128, 2, S], f8, name="e2") for _ in range(JT // 2)]

                def dn_ot(t, last):
                    for n in range(NH):
                        lo, hi = n * 512, (n + 1) * 512
                        nc.tensor.matmul(
                            dns[n], lhsT=ones8, rhs=e2s[t][:, :, lo:hi],
                            start=(t == 0), stop=last,
                            perf_mode=PM.DoubleRow,
                        )
                        nc.tensor.matmul(
                            ots[n],
                            lhsT=vt_sb[b][:, 2 * t:2 * t + 2, h * 128:(h + 1) * 128],
                            rhs=e2s[t][:, :, lo:hi],
                            start=(t == 0), stop=last,
                            perf_mode=PM.DoubleRow,
                        )
                        if last:
                            rc = rc_pool.tile([128, 512], f32, name="rc")
                            nc.vector.reciprocal_approx_fast(out=rc, in_=dns[n])
                            nc.vector.tensor_mul(
                                ao_sb[b][:, h, lo:hi], ots[n], rc
                            )

                # scores + exp run one j-tile pair ahead of dn/ot accumulation
                for jt in range(JT):
                    for n in range(NH):
                        lo, hi = n * 512, (n + 1) * 512
                        sp = mm1.tile([128, 512], f32, name="sp", tag="mm")
                        nc.tensor.matmul(
                            sp,
                            lhsT=k_sb[b][:, h, jt * 128:(jt + 1) * 128],
                            rhs=q_sb[b][:, h, lo:hi],
                            start=True,
                            stop=True,
                        )
                        if n == 1 and jt not in (0, 4):
                            nc.vector.tensor_scalar(
                                e2s[jt // 2].bitcast(u8)[:, jt % 2, lo:hi], sp,
                                SCALE * A_SCH, B_SCH,
                                op0=Alu.mult, op1=Alu.add,
                            )
                        else:
                            nc.scalar.activation(
                                out=e2s[jt // 2][:, jt % 2, lo:hi], in_=sp,
                                func=Act.Exp, scale=SCALE,
                            )
                    if jt % 2 == 1 and 5 <= jt:
                        dn_ot((jt - 5) // 2, last=False)
                dn_ot(2, last=False)
                dn_ot(3, last=True)

        def emit_proj(b):
            # batch 1: emit the ACT+GpSimd (odd) units first so GpSimd's adds
            # and end-of-block drain overlap the DVE affines of the rest
            units = [(m, n) for m in range(CT) for n in range(NH)]
            if b == 1:
                units.sort(key=lambda u: (u[0] + u[1]) % 2 == 0)
            for m, n in units:
                if True:
                    ps = mm1.tile([128, 512], f32, name="qk_ps", tag="mm")
                    for u in range(2):
                        nc.tensor.matmul(
                            ps,
                            lhsT=projT8[:, 2 * u:2 * u + 2, m * 128:(m + 1) * 128],
                            rhs=ao_sb[b][:, 2 * u:2 * u + 2, n * 512:(n + 1) * 512],
                            start=(u == 0),
                            stop=(u == 1),
                            perf_mode=PM.DoubleRow,
                        )
                    fo = fo_pool.tile([128, 512], f32, name="fo")
                    # fo = (ps/256 + projb_eff) + xn; batch 1's odd units
                    # drain via ACT+GpSimd so the kernel tail is not serial
                    # on the Vector engine
                    if b == 0 or (m + n) % 2 == 0:
                        nc.vector.affine_then_add(
                            out=fo,
                            in0=ps,
                            in1=xn_bf[m][:, b, n * 512:(n + 1) * 512],
                            scale=1.0 / 256.0,
                            bias=projb_sb[:, m:m + 1],
                        )
                    else:
                        pt = fo_pool.tile([128, 512], bf16, name="pt")
                        nc.scalar.activation(
                            out=pt, in_=ps, func=Act.Identity,
                            bias=projb_sb[:, m:m + 1], scale=1.0 / 256.0,
                        )
                        nc.gpsimd.tensor_add(
                            fo, pt, xn_bf[m][:, b, n * 512:(n + 1) * 512]
                        )
                    nc.sync.dma_start(
                        out=out_d[b, m * 128:(m + 1) * 128, n * 512:(n + 1) * 512],
                        in_=fo,
                    )

        emit_gn_stats(0)
        emit_consts_gn()
        emit_gn_rest(0, norm_on_act=True)
        emit_consts_weights()
        emit_gn_stats(1)
        emit_qkv(0, evac_on_act=True)
        emit_gn_rest(1, norm_on_act=False)
        emit_attn(0)
        emit_qkv(1)
        emit_proj(0)
        emit_attn(1)
        emit_proj(1)


def _build_nc():
    import concourse.tile as tile
    from concourse import bacc, mybir

    f32 = mybir.dt.float32
    f32r = mybir.dt.float32r
    bf16 = mybir.dt.bfloat16
    f8 = mybir.dt.float8e4
    nc = bacc.Bacc("TRN2", target_bir_lowering=False, debug=False)
    io = {
        "x": nc.dram_tensor("x", [BPC, C, S], f32, kind="ExternalInput").ap(),
        "qkvT8": nc.dram_tensor("qkvT8", [128, CT, 3 * C], f8, kind="ExternalInput").ap(),
        "projT8": nc.dram_tensor("projT8", [128, CT, C], f8, kind="ExternalInput").ap(),
        "qkvb": nc.dram_tensor("qkvb", [128, 12], f32, kind="ExternalInput").ap(),
        "gnw": nc.dram_tensor("gnw", [128, CT], f32, kind="ExternalInput").ap(),
        "gnb": nc.dram_tensor("gnb", [128, CT], f32, kind="ExternalInput").ap(),
        "projb": nc.dram_tensor("projb", [128, CT], f32, kind="ExternalInput").ap(),
        "indp": nc.dram_tensor("indp", [128, 8], f32r, kind="ExternalInput").ap(),
        "indb": nc.dram_tensor("indb", [8, 128], f32r, kind="ExternalInput").ap(),
        "out": nc.dram_tensor("out", [BPC, C, S], f32, kind="ExternalOutput").ap(),
    }
    with tile.TileContext(nc) as tc:
        _emit(tc, io)
    nc.compile()
    return nc


def get_nc():
    if "nc" not in _CACHE:
        _CACHE["nc"] = _build_nc()
    return _CACHE["nc"]


def make_const_inputs(norm_w, norm_b, qkv_w, qkv_b, proj_w, proj_b):
    """Host-side constant tensors shared by all cores."""
    import ml_dtypes

    f = np.float32
    bf = ml_dtypes.bfloat16
    qkv_w = np.asarray(qkv_w, dtype=np.float64)
    qkv_b = np.asarray(qkv_b, dtype=np.float64)
    proj_w = np.asarray(proj_w, dtype=np.float64)
    proj_b = np.asarray(proj_b, dtype=np.float64)
    proj_b = proj_b + proj_w @ qkv_b[2 * C:3 * C]   # fold v-bias
    fp8 = ml_dtypes.float8_e4m3
    # qkvT8[p, kt, o] = 16 * qkv_w[o, kt*128 + p]
    qkvT8 = np.ascontiguousarray(
        (16.0 * qkv_w.T).reshape(4, 128, 3 * C).transpose(1, 0, 2).astype(fp8)
    )
    projT8 = np.ascontiguousarray(
        (16.0 * proj_w.T).reshape(4, 128, C).transpose(1, 0, 2).astype(fp8)
    )
    qkvb = np.ascontiguousarray(qkv_b.reshape(12, 128).T, dtype=f)
    gnw = np.ascontiguousarray(norm_w.reshape(CT, 128).T, dtype=f)
    gnb = np.ascontiguousarray(norm_b.reshape(CT, 128).T, dtype=f)
    projb = np.ascontiguousarray(proj_b.reshape(CT, 128).T, dtype=f)
    indp = np.zeros((128, 8), dtype=f)
    for p in range(128):
        indp[p, p // 16] = 1.0 / 16.0
    indb = np.zeros((8, 128), dtype=f)
    for p in range(128):
        indb[p // 16, p] = 1.0
    return {
        "qkvT8": qkvT8, "projT8": projT8, "qkvb": qkvb,
        "gnw": gnw, "gnb": gnb, "projb": projb,
        "indp": indp, "indb": indb,
    }


def kernel(x, norm_w, norm_b, qkv_w, qkv_b, proj_w, proj_b, _trace=False):
    from concourse.bass_utils import run_bass_kernel_spmd

    b, c, h, w = x.shape
    assert (b, c, h * w) == (B, C, S), f"unexpected input shape {x.shape}"
    consts = make_const_inputs(norm_w, norm_b, qkv_w, qkv_b, proj_w, proj_b)
    xf = np.ascontiguousarray(x.reshape(B, C, S), dtype=np.float32)
    in_maps = [
        {"x": np.ascontiguousarray(xf[i * BPC:(i + 1) * BPC]), **consts}
        for i in range(NCORES)
    ]
    nc = get_nc()
    res = run_bass_kernel_spmd(
        nc, in_maps, core_ids=list(range(NCORES)), trace=_trace
    )
    out = np.concatenate([r["out"] for r in res.results], axis=0)
    out = out.reshape(B, C, h, w).astype(np.float32)
    if _trace:
        _CACHE["last_results"] = res
    return out

